# revision 11
# baseline (speedup 1.0000x reference)
"""Two-layer GCN (PyG GCNConv x2) on 8 Trainium2 NeuronCores.

Strategy (edge/graph partitioning per sharding hint):
- Nodes padded to 100352 = 8 cores * 98 buckets * 128; core c owns the
  contiguous node range [c*12544, (c+1)*12544) (its "col" / target range).
- Edges (+ self loops) are assigned to the core owning their target node,
  grouped by 128-node bucket, and within a bucket binned by source-node
  quartile (for int16 gather indices). Each (bucket, bin) is padded to 2304
  slots (w=0) => every bucket is exactly 72 tiles of 128 edges.
- Per edge tile, a [128 edges x 128 nodes] bf16 one-hot of the target lane
  (colrel) is built on DVE; the scatter-add segment-sum is one PE matmul
  per tile accumulating out^T[F, 128] in PSUM (contract over edges).
- h[row] gathers use the native SWDGE dma_gather (256B rows, int16 indices
  relative to a 25088-row table chunk).
- deg -> dinv = rsqrt(deg) is computed once (same graph both layers); the
  symmetric normalization is folded as table g = dinv * (x @ W) (row side)
  and a post-scale by dinv on the output (col side).
- Two AllGathers distribute the gather tables between layers; outputs are
  sliced per core and concatenated on the host.
"""
import os, sys

sys.path.insert(0, "/opt/trn_rl_repo")

STAGE = int(os.environ.get("GCN_STAGE", "9"))

import numpy as np
import ml_dtypes
from contextlib import ExitStack

import concourse.bass as bass
import concourse.bacc as bacc
import concourse.tile as tile
import concourse.mybir as mybir
from concourse.bass_utils import run_bass_kernel_spmd
from concourse.masks import make_identity

P = 128
CORES = 8
NB = 98                 # buckets per core
NPC = NB * P            # nodes per core = 12544
NPAD = CORES * NPC      # 100352
N = 100000
BINS = 4
CHUNK = NPAD // BINS    # 25088 (< 2^15, int16-safe)
BIN_T = 19              # tiles per bin
T = BINS * BIN_T        # 72 tiles per bucket
BIN_SLOTS = BIN_T * P   # 2304
ES = 64                 # gather table row = 64 f32 = 256B
F0 = 16
F1 = 8
F2 = 2
FP32 = mybir.dt.float32
BF16 = mybir.dt.bfloat16
I16 = mybir.dt.int16
AG_GROUPS = [list(range(CORES))]

_CACHE = {}


def _build():
    nc = bacc.Bacc("TRN2", target_bir_lowering=False, debug=False,
                   num_devices=CORES)
    eq = mybir.AluOpType.is_equal
    mul = mybir.AluOpType.mult
    add = mybir.AluOpType.add
    amax = mybir.AluOpType.max

    xT_d = nc.dram_tensor("xT", [F0, NPC], FP32, kind="ExternalInput")
    W1_d = nc.dram_tensor("W1", [F0, F1], FP32, kind="ExternalInput")
    W2_d = nc.dram_tensor("W2", [F1, F2], FP32, kind="ExternalInput")
    b1_d = nc.dram_tensor("b1c", [F1, 1], FP32, kind="ExternalInput")
    b2_d = nc.dram_tensor("b2c", [F2, 1], FP32, kind="ExternalInput")
    colrel_d = nc.dram_tensor("colrel", [NB, P, T], BF16, kind="ExternalInput")
    wgt_d = nc.dram_tensor("wgt", [NB, P, T], FP32, kind="ExternalInput")
    idx_d = nc.dram_tensor("idx", [NB, P, BINS, BIN_SLOTS // 16], I16,
                           kind="ExternalInput")
    xemb_d = nc.dram_tensor("xemb", [NPC, F1], FP32, kind="ExternalOutput")
    out2_d = nc.dram_tensor("out2", [NPC, F2], FP32, kind="ExternalOutput")

    with tile.TileContext(nc) as tc, ExitStack() as ctx:
        cst = ctx.enter_context(tc.tile_pool(name="cst", bufs=1))
        sb = ctx.enter_context(tc.tile_pool(name="sb", bufs=2))
        big = ctx.enter_context(tc.tile_pool(name="big", bufs=1))
        dram = ctx.enter_context(tc.tile_pool(name="dram", bufs=1, space="DRAM"))
        ps_acc = ctx.enter_context(tc.tile_pool(name="psacc", bufs=2, space="PSUM"))
        ps_tr = ctx.enter_context(tc.tile_pool(name="pstr", bufs=2, space="PSUM"))
        ps_ch = ctx.enter_context(tc.tile_pool(name="psch", bufs=3, space="PSUM"))

        # ---- internal DRAM ----
        deg_dram = dram.tile([NPC], FP32)
        s1t_dram = dram.tile([F1, NPC], FP32)
        s2t_dram = dram.tile([F2, NPC], FP32)
        g1_in = dram.tile([NPC, ES], FP32)
        g1_tab = dram.tile([NPAD, ES], FP32)
        g2_in = dram.tile([NPC, ES], FP32)
        g2_tab = dram.tile([NPAD, ES], FP32)

        # ---- constants ----
        iota_i = cst.tile([P, P], I16)
        nc.gpsimd.iota(iota_i[:], pattern=[[1, P]], base=0, channel_multiplier=0)
        iota_b = cst.tile([P, P], BF16)
        nc.vector.tensor_copy(iota_b[:], iota_i[:])
        ident = cst.tile([P, P], FP32)
        make_identity(nc, ident[:])
        W1_sb = cst.tile([F0, F1], FP32)
        nc.sync.dma_start(W1_sb[:], W1_d[:, :])
        W2_sb = cst.tile([F1, F2], FP32)
        nc.sync.dma_start(W2_sb[:], W2_d[:, :])
        b1_sb = cst.tile([F1, 1], FP32)
        nc.sync.dma_start(b1_sb[:], b1_d[:, :])
        b2_sb = cst.tile([F2, 1], FP32)
        nc.sync.dma_start(b2_sb[:], b2_d[:, :])
        ones_sb = cst.tile([1, F1], FP32)
        nc.vector.memset(ones_sb[:], 1.0)

        def load_edges(i, want_w32):
            cr = sb.tile([P, T], BF16, tag="cr")
            nc.sync.dma_start(cr[:], colrel_d[bass.ds(i, 1), :, :].squeeze(0))
            w32 = sb.tile([P, T], FP32, tag="w32")
            if want_w32:
                nc.sync.dma_start(w32[:], wgt_d[bass.ds(i, 1), :, :].squeeze(0))
            oh = sb.tile([P, T * P], BF16, tag="oh")
            nc.vector.tensor_tensor(
                out=oh[:].rearrange("p (t j) -> p t j", t=T),
                in0=iota_b[:].unsqueeze(1).to_broadcast([P, T, P]),
                in1=cr[:].unsqueeze(2).to_broadcast([P, T, P]),
                op=eq,
            )
            return cr, w32, oh

        # ================= pass 0: deg =================
        with tc.For_i(0, NB) as i:
            cr, w32, oh = load_edges(i, True)
            wb = sb.tile([P, T], BF16, tag="wb")
            nc.vector.tensor_copy(wb[:], w32[:])
            acc0 = ps_acc.tile([1, P], FP32, space="PSUM", tag="acc")
            for t in range(T):
                nc.tensor.matmul(out=acc0[:], lhsT=wb[:, t:t + 1],
                                 rhs=oh[:, t * P:(t + 1) * P],
                                 start=(t == 0), stop=(t == T - 1))
            dtmp = sb.tile([1, P], FP32, tag="dtmp")
            nc.vector.tensor_copy(dtmp[:], acc0[:])
            nc.sync.dma_start(deg_dram[bass.ts(i, P)], dtmp[:].squeeze(0))

        # ================= dinv =================
        dinv_sb = big.tile([1, NPC], FP32)
        nc.sync.dma_start(dinv_sb[:], deg_dram[:].unsqueeze(0))
        nc.vector.tensor_scalar(dinv_sb[:], dinv_sb[:], 1e-20, None, amax)
        nc.scalar.activation(dinv_sb[:], dinv_sb[:],
                             mybir.ActivationFunctionType.Sqrt)
        nc.vector.reciprocal(dinv_sb[:], dinv_sb[:])

        # chunks of 4 buckets (last: 2)
        chunks = [(c * 512, min(512, NPC - c * 512)) for c in range((NPC + 511) // 512)]

        # ================= g1 table: dinv * (x @ W1) =================
        for n0, ncol in (chunks if STAGE >= 1 else []):
            xt = sb.tile([F0, 512], FP32, tag="xt")
            nc.sync.dma_start(xt[:, :ncol], xT_d[:, n0:n0 + ncol])
            h1p = ps_ch.tile([F1, 512], FP32, space="PSUM", tag="chp")
            nc.tensor.matmul(out=h1p[:, :ncol], lhsT=W1_sb[:], rhs=xt[:, :ncol],
                             start=True, stop=True)
            drp = ps_ch.tile([F1, 512], FP32, space="PSUM", tag="chp")
            nc.tensor.matmul(out=drp[:, :ncol], lhsT=ones_sb[:],
                             rhs=dinv_sb[:, n0:n0 + ncol], start=True, stop=True)
            drs = sb.tile([F1, 512], FP32, tag="drs")
            nc.vector.tensor_copy(drs[:, :ncol], drp[:, :ncol])
            g1t = sb.tile([F1, 512], FP32, tag="g1t")
            nc.vector.tensor_tensor(out=g1t[:, :ncol], in0=h1p[:, :ncol],
                                    in1=drs[:, :ncol], op=mul)
            for k in range(ncol // P):
                trp = ps_tr.tile([P, F1], FP32, space="PSUM", tag="trp")
                nc.tensor.transpose(trp[:], g1t[:, k * P:(k + 1) * P],
                                    ident[:F1, :F1])
                gnm = sb.tile([P, F1], FP32, tag="gnm")
                nc.vector.tensor_copy(gnm[:], trp[:])
                r0 = n0 + k * P
                nc.sync.dma_start(g1_in[r0:r0 + P, 0:F1], gnm[:])

        if STAGE >= 2:
            nc.gpsimd.collective_compute(
                "AllGather", mybir.AluOpType.bypass, replica_groups=AG_GROUPS,
                ins=[g1_in.opt()], outs=[g1_tab.opt()])

        # ================= pass/post helper =================
        def edge_pass(F, tab, st_dram):
            with tc.For_i(0, NB) as i:
                cr, w32, oh = load_edges(i, True)
                ix = sb.tile([P, BINS * (BIN_SLOTS // 16)], I16, tag="ix")
                nc.sync.dma_start(
                    ix[:].rearrange("p (a b) -> p a b", a=BINS),
                    idx_d[bass.ds(i, 1), :, :, :].squeeze(0))
                gath = sb.tile([P, T, ES], FP32, tag="gath")
                for k in range(BINS):
                    nc.gpsimd.dma_gather(
                        gath[:, k * BIN_T:(k + 1) * BIN_T, :],
                        tab[k * CHUNK:(k + 1) * CHUNK, :],
                        ix[:, k * (BIN_SLOTS // 16):(k + 1) * (BIN_SLOTS // 16)],
                        BIN_SLOTS, BIN_SLOTS, ES)
                msg = sb.tile([P, T * F], BF16, tag="msg")
                nc.vector.tensor_tensor(
                    out=msg[:].rearrange("p (t f) -> p t f", t=T),
                    in0=gath[:, :, 0:F],
                    in1=w32[:].unsqueeze(2).to_broadcast([P, T, F]),
                    op=mul)
                acc = ps_acc.tile([F, P], FP32, space="PSUM", tag="acc")
                for t in range(T):
                    nc.tensor.matmul(out=acc[:], lhsT=msg[:, t * F:(t + 1) * F],
                                     rhs=oh[:, t * P:(t + 1) * P],
                                     start=(t == 0), stop=(t == T - 1))
                stmp = sb.tile([F, P], FP32, tag="stmp")
                nc.vector.tensor_copy(stmp[:], acc[:])
                nc.sync.dma_start(st_dram[:, bass.ts(i, P)], stmp[:])

        def post_pass(F, st_dram, b_sb, nm_tag, do_g2):
            nm = big.tile([P, NB * F], FP32, tag=nm_tag, name=nm_tag)
            if do_g2:
                g2nm = big.tile([P, NB * F2], FP32, tag="g2nm", name="g2nm")
            else:
                g2nm = None
            for n0, ncol in chunks:
                st = sb.tile([F, 512], FP32, tag="st")
                nc.sync.dma_start(st[:, :ncol], st_dram[:, n0:n0 + ncol])
                drp = ps_ch.tile([F1, 512], FP32, space="PSUM", tag="chp")
                nc.tensor.matmul(out=drp[:, :ncol], lhsT=ones_sb[:],
                                 rhs=dinv_sb[:, n0:n0 + ncol], start=True, stop=True)
                xe = sb.tile([F, 512], FP32, tag="xe")
                nc.vector.tensor_tensor(out=xe[:, :ncol], in0=st[:, :ncol],
                                        in1=drp[:F, :ncol], op=mul)
                nc.vector.tensor_scalar(xe[:, :ncol], xe[:, :ncol], b_sb[:], None, add)
                for k in range(ncol // P):
                    trp = ps_tr.tile([P, F1], FP32, space="PSUM", tag="trp")
                    nc.tensor.transpose(trp[:, :F], xe[:, k * P:(k + 1) * P],
                                        ident[:F, :F])
                    b = (n0 // P) + k
                    nc.vector.tensor_copy(nm[:, b * F:(b + 1) * F], trp[:, :F])
                if do_g2:
                    hT = sb.tile([F, 512], FP32, tag="hT")
                    nc.vector.tensor_scalar(hT[:, :ncol], xe[:, :ncol], 0.0, None, amax)
                    g2p = ps_ch.tile([F2, 512], FP32, space="PSUM", tag="chp")
                    nc.tensor.matmul(out=g2p[:, :ncol], lhsT=W2_sb[:],
                                     rhs=hT[:, :ncol], start=True, stop=True)
                    drs2 = sb.tile([F2, 512], FP32, tag="drs2")
                    nc.vector.tensor_copy(drs2[:, :ncol], drp[:F2, :ncol])
                    g2t = sb.tile([F2, 512], FP32, tag="g2t")
                    nc.vector.tensor_tensor(out=g2t[:, :ncol], in0=g2p[:, :ncol],
                                            in1=drs2[:, :ncol], op=mul)
                    for k in range(ncol // P):
                        trp2 = ps_tr.tile([P, F1], FP32, space="PSUM", tag="trp")
                        nc.tensor.transpose(trp2[:, :F2], g2t[:, k * P:(k + 1) * P],
                                            ident[:F2, :F2])
                        b = (n0 // P) + k
                        nc.vector.tensor_copy(g2nm[:, b * F2:(b + 1) * F2],
                                              trp2[:, :F2])
            return nm, g2nm

        # ================= pass 1 + post =================
        if STAGE >= 3:
            edge_pass(F1, g1_tab, s1t_dram)
        if STAGE >= 4:
            xemb_nm, g2nm = post_pass(F1, s1t_dram, b1_sb, "xembnm", True)
            nc.sync.dma_start(
                xemb_d[:, :].rearrange("(b p) f -> p b f", p=P),
                xemb_nm[:].rearrange("p (b f) -> p b f", f=F1))
            nc.sync.dma_start(
                g2_in[:, 0:F2].rearrange("(b p) f -> p b f", p=P),
                g2nm[:].rearrange("p (b f) -> p b f", f=F2))
        if STAGE >= 5:
            nc.gpsimd.collective_compute(
                "AllGather", mybir.AluOpType.bypass, replica_groups=AG_GROUPS,
                ins=[g2_in.opt()], outs=[g2_tab.opt()])

        # ================= pass 2 + post =================
        if STAGE >= 6:
            edge_pass(F2, g2_tab, s2t_dram)
        if STAGE >= 7:
            out2_nm, _ = post_pass(F2, s2t_dram, b2_sb, "out2nm", False)
            nc.sync.dma_start(
                out2_d[:, :].rearrange("(b p) f -> p b f", p=P),
                out2_nm[:].rearrange("p (b f) -> p b f", f=F2))

    nc.compile()
    return nc


def _prep(x, edge_index, edge_weight):
    row = np.asarray(edge_index[0], dtype=np.int64).astype(np.int32)
    col = np.asarray(edge_index[1], dtype=np.int64).astype(np.int32)
    loop = np.arange(N, dtype=np.int32)
    row = np.concatenate([row, loop])
    col = np.concatenate([col, loop])
    w = np.concatenate([np.asarray(edge_weight, np.float32),
                        np.ones(N, np.float32)])

    bucketg = (col >> 7).astype(np.int64)          # 0..783
    binid = (row // CHUNK).astype(np.int64)        # 0..3
    key = bucketg * BINS + binid
    order = np.argsort(key, kind="stable")
    row, col, w, key = row[order], col[order], w[order], key[order]

    nbins_total = (NPAD // P) * BINS
    counts = np.bincount(key, minlength=nbins_total)
    if counts.max() > BIN_SLOTS:
        raise RuntimeError(f"bin overflow: {counts.max()} > {BIN_SLOTS}")
    starts = np.zeros(nbins_total + 1, np.int64)
    np.cumsum(counts, out=starts[1:])
    within = np.arange(len(key), dtype=np.int64) - starts[key]
    slot = key * BIN_SLOTS + within

    total = nbins_total * BIN_SLOTS
    colrel_p = np.zeros(total, np.int16)
    w_p = np.zeros(total, np.float32)
    idxrel_p = np.zeros(total, np.int16)
    colrel_p[slot] = (col & 127).astype(np.int16)
    w_p[slot] = w
    idxrel_p[slot] = (row % CHUNK).astype(np.int16)

    # [core, NB, BINS, BIN_T, P] -> device layouts
    cr5 = colrel_p.reshape(CORES, NB, BINS, BIN_T, P)
    w5 = w_p.reshape(CORES, NB, BINS, BIN_T, P)
    colrel_dev = np.ascontiguousarray(
        cr5.transpose(0, 1, 4, 2, 3).reshape(CORES, NB, P, T)
    ).astype(ml_dtypes.bfloat16)
    w_dev = np.ascontiguousarray(w5.transpose(0, 1, 4, 2, 3).reshape(CORES, NB, P, T))
    ix5 = idxrel_p.reshape(CORES, NB, BINS, BIN_SLOTS // 16, 16)
    idx_dev = np.ascontiguousarray(
        np.tile(ix5.transpose(0, 1, 2, 4, 3), (1, 1, 1, CORES, 1))
        .transpose(0, 1, 3, 2, 4))  # [core, NB, P, BINS, n/16]

    xpad = np.zeros((NPAD, F0), np.float32)
    xpad[:N] = np.asarray(x, np.float32)
    xT = np.ascontiguousarray(xpad.T)
    return colrel_dev, w_dev, idx_dev, xT


def kernel(x, edge_index, edge_weight, W1, b1, W2, b2):
    if "nc" not in _CACHE:
        _CACHE["nc"] = _build()
    nc = _CACHE["nc"]

    colrel_dev, w_dev, idx_dev, xT = _prep(x, edge_index, edge_weight)
    W1f = np.asarray(W1, np.float32)
    W2f = np.asarray(W2, np.float32)
    b1c = np.asarray(b1, np.float32).reshape(F1, 1)
    b2c = np.asarray(b2, np.float32).reshape(F2, 1)

    in_maps = []
    for c in range(CORES):
        in_maps.append({
            "xT": np.ascontiguousarray(xT[:, c * NPC:(c + 1) * NPC]),
            "W1": W1f, "W2": W2f, "b1c": b1c, "b2c": b2c,
            "colrel": colrel_dev[c], "wgt": w_dev[c], "idx": idx_dev[c],
        })
    res = run_bass_kernel_spmd(nc, in_maps, core_ids=list(range(CORES)))
    xemb = np.concatenate([r["xemb"] for r in res.results], axis=0)[:N]
    out2 = np.concatenate([r["out2"] for r in res.results], axis=0)[:N]
    return out2, xemb
